# revision 3
# baseline (speedup 1.0000x reference)
"""ComplexMixture Trainium2 kernel.

Computes, for each batch b of input_real/input_imag [B, S, D]:
    out_real[b] = (R^T R + I^T I) / S          (symmetric   [D, D])
    out_imag[b] = (R^T I - (R^T I)^T) / S      (antisym     [D, D])
with B=32, S=8192, D=64.

Strategy: data-parallel over batch across 8 NeuronCores (4 batches/core).
Host packs Z = [R | I] ([S, 2D]) per batch.  Per batch, one fused Gram
matmul G = Z^T Z accumulated over 64 K-tiles of 128 sequence rows into a
[128, 128] PSUM tile.  G = [[rr, ri], [ri^T, ii]].  A tiny "shift" matmul
H = J64^T G (J64 = rows 64:128 of the 128-identity) moves the bottom 64
partitions of G up so the block combines are plain elementwise ops:
    out_real = G[0:64, 0:64] + H[:, 64:128]
    out_imag = G[0:64, 64:128] - H[:, 0:64]
(1/S is folded into the PSUM->SBUF copy of G).
"""

import os
import numpy as np

import concourse.bass as bass
import concourse.tile as tile
from concourse import bacc, mybir
from concourse.bass_utils import run_bass_kernel_spmd

B, S, D = 32, 8192, 64
D2 = 2 * D                  # packed feature width (R|I)
N_CORES = 8
BPC = B // N_CORES          # batches per core
P = 128                     # partitions / K-tile size
T = S // P                  # K-tiles per batch
INV_S = 1.0 / S

_NC_CACHE = {}
LAST_RESULTS = None         # BassKernelResults of the most recent run


def _build_nc():
    nc = bacc.Bacc("TRN2", target_bir_lowering=False, debug=False)

    xz = nc.dram_tensor("xz", [BPC, S, D2], mybir.dt.float32, kind="ExternalInput")
    j64 = nc.dram_tensor("j64", [P, D], mybir.dt.float32, kind="ExternalInput")
    out = nc.dram_tensor("out", [BPC, 2, D, D], mybir.dt.float32, kind="ExternalOutput")

    with tile.TileContext(nc) as tc:
        with (
            tc.tile_pool(name="consts", bufs=1) as consts,
            tc.tile_pool(name="zpool", bufs=3) as zpool,
            tc.tile_pool(name="gpool", bufs=2) as gpool,
            tc.tile_pool(name="opool", bufs=2) as opool,
            tc.tile_pool(name="psg", bufs=2, space="PSUM") as psg,
            tc.tile_pool(name="psh", bufs=2, space="PSUM") as psh,
        ):
            j64_sb = consts.tile([P, D], mybir.dt.float32)
            nc.sync.dma_start(out=j64_sb[:], in_=j64[:])

            for b in range(BPC):
                # z[p, t, c] = Z[b, p*T + t, c]; K-tile t = rows {p*T + t}.
                z = zpool.tile([P, T, D2], mybir.dt.float32)
                nc.sync.dma_start(
                    out=z[:], in_=xz[b].rearrange("(p t) c -> p t c", p=P)
                )

                g_ps = psg.tile([P, P], mybir.dt.float32)
                for t in range(T):
                    zt = z[:, t, :]
                    nc.tensor.matmul(
                        g_ps[:], zt, zt, start=(t == 0), stop=(t == T - 1)
                    )

                g_sb = gpool.tile([P, P], mybir.dt.float32)
                nc.vector.tensor_scalar_mul(g_sb[:], g_ps[:], INV_S)

                h_ps = psh.tile([D, P], mybir.dt.float32)
                nc.tensor.matmul(h_ps[:], j64_sb[:], g_sb[:], start=True, stop=True)

                o_sb = opool.tile([D, 2, D], mybir.dt.float32)
                nc.vector.tensor_add(o_sb[:, 0], g_sb[0:D, 0:D], h_ps[:, D : 2 * D])
                nc.vector.tensor_sub(o_sb[:, 1], g_sb[0:D, D : 2 * D], h_ps[:, 0:D])

                nc.sync.dma_start(out=out[b, 0], in_=o_sb[:, 0])
                nc.sync.dma_start(out=out[b, 1], in_=o_sb[:, 1])

    nc.compile()
    return nc


def _j64_host():
    j = np.zeros((P, D), np.float32)
    j[D + np.arange(D), np.arange(D)] = 1.0
    return j


def kernel(input_real, input_imag):
    global LAST_RESULTS
    xr = np.asarray(input_real, dtype=np.float32)
    xi = np.asarray(input_imag, dtype=np.float32)
    assert xr.shape == (B, S, D) and xi.shape == (B, S, D)

    xz = np.concatenate([xr, xi], axis=2)  # [B, S, 2D], contiguous

    if "nc" not in _NC_CACHE:
        _NC_CACHE["nc"] = _build_nc()
    nc = _NC_CACHE["nc"]

    j64 = _j64_host()
    in_maps = [
        {"xz": xz[c * BPC : (c + 1) * BPC], "j64": j64} for c in range(N_CORES)
    ]
    tmpdir = os.environ.get("BASS_TMPDIR") or None
    res = run_bass_kernel_spmd(
        nc, in_maps, core_ids=list(range(N_CORES)), tmpdir=tmpdir
    )
    LAST_RESULTS = res

    outs = np.stack([res.results[c]["out"] for c in range(N_CORES)])  # [8,BPC,2,D,D]
    out = outs.reshape(B, 2, D, D)
    return np.ascontiguousarray(out[:, 0]), np.ascontiguousarray(out[:, 1])


# revision 6
# speedup vs baseline: 1.1630x; 1.1630x over previous
"""ComplexMixture Trainium2 kernel.

Computes, for each batch b of input_real/input_imag [B, S, D]:
    out_real[b] = (R^T R + I^T I) / S          (symmetric   [D, D])
    out_imag[b] = (R^T I - (R^T I)^T) / S      (antisym     [D, D])
with B=32, S=8192, D=64.

Strategy: data-parallel over batch across 8 NeuronCores (4 batches/core).
Host packs Z = [R | I] ([S, 2D]) per batch.  Per batch we need the Gram
matrix G = Z^T Z ([128, 128]); G = [[rr, ri], [ri^T, ii]].  A tiny
"shift" matmul H = J64^T G (J64 = rows 64:128 of the 128-identity) moves
the bottom 64 partitions of G up so the block combines are elementwise:
    out_real = G[0:64, 0:64] + H[:, 64:128]
    out_imag = G[0:64, 64:128] - H[:, 0:64]
(1/S is folded into the PSUM->SBUF copy of G).

Variants (VARIANT):
  "f32r_pair": batches are processed in pairs.  The host interleaves the
    pair at 128-float granularity so SBUF k-tiles hold [Z_a | Z_b]
    ([128, 256] contiguous).  G_a and G_b come from two N=256 float32r
    matmuls per k-tile (out[j] = Z_j^T [Z_a|Z_b]); at N>=256 float32r
    runs at 1 cycle/row (4x the fp32 rate), at the cost of computing a
    garbage cross-batch half that is simply ignored.
  "fp32": plain fp32, one N=128 matmul per k-tile (4 cycles/row).
Both stream inputs in 2048-row chunks (fully contiguous DMAs) so the PE
starts after the first ~1-2 MB rather than after the whole batch.
"""

import os
import numpy as np

import concourse.bass as bass
import concourse.tile as tile
from concourse import bacc, mybir
from concourse.bass_utils import run_bass_kernel_spmd

B, S, D = 32, 8192, 64
D2 = 2 * D                  # packed feature width (R|I)
N_CORES = 8
BPC = B // N_CORES          # batches per core
P = 128                     # partitions / K-tile size
T = S // P                  # K-tiles per batch
NCHUNK = 4                  # DMA chunks per batch(-pair)
TC = T // NCHUNK            # K-tiles per chunk
ROWS_C = S // NCHUNK        # sequence rows per chunk
INV_S = 1.0 / S

VARIANT = os.environ.get("KERNEL_VARIANT", "f32r_pair")

_NC_CACHE = {}
LAST_RESULTS = None         # BassKernelResults of the most recent run


def _fixup(nc, gpool, opool, psh, j64_sb, g_ps_slice, out, b):
    """Extract out_real/out_imag for one batch from its [128,128] G region."""
    g_sb = gpool.tile([P, P], mybir.dt.float32)
    nc.vector.tensor_scalar_mul(g_sb[:], g_ps_slice, INV_S)

    h_ps = psh.tile([D, P], mybir.dt.float32)
    nc.tensor.matmul(h_ps[:], j64_sb[:], g_sb[:], start=True, stop=True)

    o_sb = opool.tile([D, 2, D], mybir.dt.float32)
    nc.vector.tensor_add(o_sb[:, 0], g_sb[0:D, 0:D], h_ps[:, D : 2 * D])
    nc.vector.tensor_sub(o_sb[:, 1], g_sb[0:D, D : 2 * D], h_ps[:, 0:D])

    nc.scalar.dma_start(out=out[b, 0], in_=o_sb[:, 0])
    nc.scalar.dma_start(out=out[b, 1], in_=o_sb[:, 1])


def _build_nc_f32r_pair():
    """Pairs of batches; k-tiles hold [Z_a | Z_b] [128, 256] float32r."""
    nc = bacc.Bacc("TRN2", target_bir_lowering=False, debug=False)

    NP = BPC // 2  # batch pairs per core
    # xz[pair, chunk, p, t, j, c] = Z[2*pair+j, chunk*ROWS_C + p*TC + t, c]
    xz = nc.dram_tensor(
        "xz", [NP, NCHUNK, P, TC, 2, D2], mybir.dt.float32r, kind="ExternalInput"
    )
    j64 = nc.dram_tensor("j64", [P, D], mybir.dt.float32, kind="ExternalInput")
    out = nc.dram_tensor("out", [BPC, 2, D, D], mybir.dt.float32, kind="ExternalOutput")

    with tile.TileContext(nc) as tc:
        with (
            tc.tile_pool(name="consts", bufs=1) as consts,
            tc.tile_pool(name="zpool", bufs=2 * NCHUNK) as zpool,
            tc.tile_pool(name="gpool", bufs=2) as gpool,
            tc.tile_pool(name="opool", bufs=2) as opool,
            tc.tile_pool(name="psg", bufs=4, space="PSUM") as psg,
            tc.tile_pool(name="psh", bufs=2, space="PSUM") as psh,
        ):
            j64_sb = consts.tile([P, D], mybir.dt.float32)
            nc.sync.dma_start(out=j64_sb[:], in_=j64[:])

            for pair in range(NP):
                zc = []
                for c in range(NCHUNK):
                    z = zpool.tile([P, TC, 2, D2], mybir.dt.float32r, name=f"z_{pair}_{c}", tag="z")
                    eng = nc.sync if c % 2 == 0 else nc.scalar
                    eng.dma_start(out=z[:], in_=xz[pair, c])
                    zc.append(z)

                g_ps = [
                    psg.tile([P, 2 * P], mybir.dt.float32, name=f"g_ps_{pair}_{j}", tag="g")
                    for j in range(2)
                ]
                for c in range(NCHUNK):
                    for t in range(TC):
                        pair_rhs = zc[c][:, t, :, :]   # [128, 256] contiguous
                        first = c == 0 and t == 0
                        last = c == NCHUNK - 1 and t == TC - 1
                        for j in range(2):
                            nc.tensor.matmul(
                                g_ps[j][:],
                                zc[c][:, t, j, :],     # lhsT [128, 128]
                                pair_rhs,              # rhs  [128, 256]
                                start=first,
                                stop=last,
                            )

                for j in range(2):
                    _fixup(
                        nc, gpool, opool, psh, j64_sb,
                        g_ps[j][:, j * P : (j + 1) * P],
                        out, 2 * pair + j,
                    )

    nc.compile()
    return nc


def _build_nc_fp32():
    """Plain fp32, one batch at a time, chunked loads."""
    nc = bacc.Bacc("TRN2", target_bir_lowering=False, debug=False)

    # xz[b, chunk, p, t, c] = Z[b, chunk*ROWS_C + p*TC + t, c]
    xz = nc.dram_tensor(
        "xz", [BPC, NCHUNK, P, TC, D2], mybir.dt.float32, kind="ExternalInput"
    )
    j64 = nc.dram_tensor("j64", [P, D], mybir.dt.float32, kind="ExternalInput")
    out = nc.dram_tensor("out", [BPC, 2, D, D], mybir.dt.float32, kind="ExternalOutput")

    with tile.TileContext(nc) as tc:
        with (
            tc.tile_pool(name="consts", bufs=1) as consts,
            tc.tile_pool(name="zpool", bufs=3 * NCHUNK) as zpool,
            tc.tile_pool(name="gpool", bufs=2) as gpool,
            tc.tile_pool(name="opool", bufs=2) as opool,
            tc.tile_pool(name="psg", bufs=2, space="PSUM") as psg,
            tc.tile_pool(name="psh", bufs=2, space="PSUM") as psh,
        ):
            j64_sb = consts.tile([P, D], mybir.dt.float32)
            nc.sync.dma_start(out=j64_sb[:], in_=j64[:])

            for b in range(BPC):
                zc = []
                for c in range(NCHUNK):
                    z = zpool.tile([P, TC, D2], mybir.dt.float32, name=f"z_{b}_{c}", tag="z")
                    eng = nc.sync if c % 2 == 0 else nc.scalar
                    eng.dma_start(out=z[:], in_=xz[b, c])
                    zc.append(z)

                g_ps = psg.tile([P, P], mybir.dt.float32)
                for c in range(NCHUNK):
                    for t in range(TC):
                        zt = zc[c][:, t, :]
                        nc.tensor.matmul(
                            g_ps[:], zt, zt,
                            start=(c == 0 and t == 0),
                            stop=(c == NCHUNK - 1 and t == TC - 1),
                        )

                _fixup(nc, gpool, opool, psh, j64_sb, g_ps[:], out, b)

    nc.compile()
    return nc


def _j64_host():
    j = np.zeros((P, D), np.float32)
    j[D + np.arange(D), np.arange(D)] = 1.0
    return j


def _prep_f32r_pair(xz):
    """[B, S, D2] -> per-core [NP, NCHUNK, P, TC, 2, D2] pair-interleaved."""
    # (core, pair, j, chunk, p, t, c) -> (core, pair, chunk, p, t, j, c)
    a = xz.reshape(N_CORES, BPC // 2, 2, NCHUNK, P, TC, D2)
    return np.ascontiguousarray(a.transpose(0, 1, 3, 4, 5, 2, 6))


def _prep_fp32(xz):
    """[B, S, D2] -> per-core [BPC, NCHUNK, P, TC, D2] (pure reshape)."""
    return np.ascontiguousarray(xz.reshape(N_CORES, BPC, NCHUNK, P, TC, D2))


def kernel(input_real, input_imag):
    global LAST_RESULTS
    xr = np.asarray(input_real, dtype=np.float32)
    xi = np.asarray(input_imag, dtype=np.float32)
    assert xr.shape == (B, S, D) and xi.shape == (B, S, D)

    xz = np.concatenate([xr, xi], axis=2)  # [B, S, 2D]

    if VARIANT == "f32r_pair":
        build, prep = _build_nc_f32r_pair, _prep_f32r_pair
    elif VARIANT == "fp32":
        build, prep = _build_nc_fp32, _prep_fp32
    else:
        raise ValueError(f"unknown VARIANT {VARIANT}")

    key = ("nc", VARIANT)
    if key not in _NC_CACHE:
        _NC_CACHE[key] = build()
    nc = _NC_CACHE[key]

    xz_cores = prep(xz)
    j64 = _j64_host()
    in_maps = [{"xz": xz_cores[c], "j64": j64} for c in range(N_CORES)]
    tmpdir = os.environ.get("BASS_TMPDIR") or None
    res = run_bass_kernel_spmd(
        nc, in_maps, core_ids=list(range(N_CORES)), tmpdir=tmpdir
    )
    LAST_RESULTS = res

    outs = np.stack([res.results[c]["out"] for c in range(N_CORES)])  # [8,BPC,2,D,D]
    out = outs.reshape(B, 2, D, D)
    return np.ascontiguousarray(out[:, 0]), np.ascontiguousarray(out[:, 1])


# revision 7
# speedup vs baseline: 2.0683x; 1.7784x over previous
"""ComplexMixture Trainium2 kernel.

Computes, for each batch b of input_real/input_imag [B, S, D]:
    out_real[b] = (R^T R + I^T I) / S          (symmetric   [D, D])
    out_imag[b] = (R^T I - (R^T I)^T) / S      (antisym     [D, D])
with B=32, S=8192, D=64.

Strategy: data-parallel over batch across 8 NeuronCores (4 batches/core).
Host packs Z = [R | I] ([S, 2D]) per batch; all per-batch outputs derive
from the Gram matrix G = Z^T Z ([128, 128]) = [[rr, ri], [ri^T, ii]].

Given (any) G in SBUF, a tiny "shift" matmul H = J64^T G (J64 = rows
64:128 of the 128-identity) moves the bottom 64 partitions of G up so the
block combines are elementwise:
    out_real = G[0:64, 0:64] + H[:, 64:128]
    out_imag = G[0:64, 64:128] - H[:, 0:64]

Variants (VARIANT):
  "fp16hl" (default, fp32-grade accuracy): host splits Z = Zh + Zl/LSCALE
    with Zh = fp16(Z), Zl = fp16((Z - Zh) * LSCALE) (scaled so Zl avoids
    fp16 subnormals).  Then
        G = Zh^T Zh + (Zh^T Zl + Zl^T Zh)/LSCALE + O(2^-22)
          = A + (C + C^T)/LSCALE,   A = Zh^T Zh, C = Zh^T Zl.
    A and C come from ONE N=256 fp16 matmul per k-tile (rhs = [Zh|Zl],
    1 cycle/row, weights Zh loaded once); C^T is one PE transpose per
    batch.  Cost ~2x a plain bf16 Gram, ~2x cheaper than fp32.
  "fp16" (fastest, ~6e-4): single fp16 Gram, half the DMA bytes.
  "fp32": plain fp32 (4 cycles/row), exact.
  "f32r_pair": float32r with batch-paired N=256 rhs (measured: same
    throughput as fp32, ~1e-4 error; kept for reference).

All variants stream inputs in ~1 MiB fully-contiguous chunks, issued on
the Sync HWDGE ring only (FIFO per ring -> chunks complete in order, so
the PE starts after the first chunk, not after the whole batch).  Small
transfers (consts, outputs) ride the Scalar ring.
"""

import os
import numpy as np

import concourse.bass as bass
import concourse.tile as tile
from concourse import bacc, mybir
from concourse.bass_utils import run_bass_kernel_spmd

B, S, D = 32, 8192, 64
D2 = 2 * D                  # packed feature width (R|I)
N_CORES = 8
BPC = B // N_CORES          # batches per core
P = 128                     # partitions / K-tile size
T = S // P                  # K-tiles per batch
INV_S = 1.0 / S
LSCALE = 2048.0             # fp16 lo-part scale (2^11)

VARIANT = os.environ.get("KERNEL_VARIANT", "fp16hl")

_NC_CACHE = {}
LAST_RESULTS = None         # BassKernelResults of the most recent run


def _shift_combine(nc, gpool, opool, psh, j64_sb, g_sb, out, b):
    """Given scaled G in SBUF ([128,128] f32), emit out_real/out_imag."""
    h_ps = psh.tile([D, P], mybir.dt.float32)
    nc.tensor.matmul(h_ps[:], j64_sb[:], g_sb[:], start=True, stop=True)

    o_sb = opool.tile([D, 2, D], mybir.dt.float32)
    nc.vector.tensor_add(o_sb[:, 0], g_sb[0:D, 0:D], h_ps[:, D : 2 * D])
    nc.vector.tensor_sub(o_sb[:, 1], g_sb[0:D, D : 2 * D], h_ps[:, 0:D])

    nc.scalar.dma_start(out=out[b, 0], in_=o_sb[:, 0])
    nc.scalar.dma_start(out=out[b, 1], in_=o_sb[:, 1])


def _build_nc_fp16hl():
    NCHUNK = 4
    TC = T // NCHUNK
    nc = bacc.Bacc("TRN2", target_bir_lowering=False, debug=False)

    # xz[b, chunk, p, t, hl, c] = Z{h,l}[b, chunk*(S/NCHUNK) + p*TC + t, c]
    xz = nc.dram_tensor(
        "xz", [BPC, NCHUNK, P, TC, 2, D2], mybir.dt.float16, kind="ExternalInput"
    )
    j64 = nc.dram_tensor("j64", [P, D], mybir.dt.float32, kind="ExternalInput")
    id128 = nc.dram_tensor("id128", [P, P], mybir.dt.float32, kind="ExternalInput")
    out = nc.dram_tensor("out", [BPC, 2, D, D], mybir.dt.float32, kind="ExternalOutput")

    with tile.TileContext(nc) as tc:
        with (
            tc.tile_pool(name="consts", bufs=1) as consts,
            tc.tile_pool(name="zpool", bufs=3 * NCHUNK) as zpool,
            tc.tile_pool(name="gpool", bufs=4) as gpool,
            tc.tile_pool(name="opool", bufs=2) as opool,
            tc.tile_pool(name="psg", bufs=2, space="PSUM") as psg,
            tc.tile_pool(name="psct", bufs=2, space="PSUM") as psct,
            tc.tile_pool(name="psh", bufs=2, space="PSUM") as psh,
        ):
            j64_sb = consts.tile([P, D], mybir.dt.float32)
            nc.scalar.dma_start(out=j64_sb[:], in_=j64[:])
            id_sb = consts.tile([P, P], mybir.dt.float32)
            nc.scalar.dma_start(out=id_sb[:], in_=id128[:])

            for b in range(BPC):
                zc = []
                for c in range(NCHUNK):
                    z = zpool.tile(
                        [P, TC, 2, D2], mybir.dt.float16, name=f"z_{b}_{c}", tag="z"
                    )
                    nc.sync.dma_start(out=z[:], in_=xz[b, c])
                    zc.append(z)

                # g1 = Zh^T [Zh | Zl]:  A = g1[:, :128] = hh, C = g1[:, 128:] = hl
                g1_ps = psg.tile([P, 2 * P], mybir.dt.float32)
                for c in range(NCHUNK):
                    for t in range(TC):
                        nc.tensor.matmul(
                            g1_ps[:],
                            zc[c][:, t, 0, :],   # lhsT = Zh_t [128, 128]
                            zc[c][:, t, :, :],   # rhs  = [Zh_t | Zl_t] [128, 256]
                            start=(c == 0 and t == 0),
                            stop=(c == NCHUNK - 1 and t == TC - 1),
                        )

                # cs = C * (inv_s / LSCALE)
                cs = gpool.tile([P, P], mybir.dt.float32, name=f"cs_{b}", tag="cs")
                nc.vector.tensor_scalar_mul(cs[:], g1_ps[:, P : 2 * P], INV_S / LSCALE)
                # ct = cs^T (PE transpose; already scaled)
                ct_ps = psct.tile([P, P], mybir.dt.float32)
                nc.tensor.transpose(ct_ps[:], cs[:], id_sb[:])
                # g = A*inv_s + cs + ct   (scaled G)
                g_sb = gpool.tile([P, P], mybir.dt.float32, name=f"g_sb_{b}", tag="g")
                nc.vector.scalar_tensor_tensor(
                    out=g_sb[:],
                    in0=g1_ps[:, 0:P],
                    scalar=INV_S,
                    in1=cs[:],
                    op0=mybir.AluOpType.mult,
                    op1=mybir.AluOpType.add,
                )
                g2_sb = gpool.tile([P, P], mybir.dt.float32, name=f"g2_sb_{b}", tag="g2")
                nc.vector.tensor_add(g2_sb[:], g_sb[:], ct_ps[:])

                _shift_combine(nc, gpool, opool, psh, j64_sb, g2_sb, out, b)

    nc.compile()
    return nc


def _build_nc_fp16():
    NCHUNK = 2
    TC = T // NCHUNK
    nc = bacc.Bacc("TRN2", target_bir_lowering=False, debug=False)

    xz = nc.dram_tensor(
        "xz", [BPC, NCHUNK, P, TC, D2], mybir.dt.float16, kind="ExternalInput"
    )
    j64 = nc.dram_tensor("j64", [P, D], mybir.dt.float32, kind="ExternalInput")
    out = nc.dram_tensor("out", [BPC, 2, D, D], mybir.dt.float32, kind="ExternalOutput")

    with tile.TileContext(nc) as tc:
        with (
            tc.tile_pool(name="consts", bufs=1) as consts,
            tc.tile_pool(name="zpool", bufs=3 * NCHUNK) as zpool,
            tc.tile_pool(name="gpool", bufs=2) as gpool,
            tc.tile_pool(name="opool", bufs=2) as opool,
            tc.tile_pool(name="psg", bufs=2, space="PSUM") as psg,
            tc.tile_pool(name="psh", bufs=2, space="PSUM") as psh,
        ):
            j64_sb = consts.tile([P, D], mybir.dt.float32)
            nc.scalar.dma_start(out=j64_sb[:], in_=j64[:])

            for b in range(BPC):
                zc = []
                for c in range(NCHUNK):
                    z = zpool.tile(
                        [P, TC, D2], mybir.dt.float16, name=f"z_{b}_{c}", tag="z"
                    )
                    nc.sync.dma_start(out=z[:], in_=xz[b, c])
                    zc.append(z)

                g_ps = psg.tile([P, P], mybir.dt.float32)
                for c in range(NCHUNK):
                    for t in range(TC):
                        zt = zc[c][:, t, :]
                        nc.tensor.matmul(
                            g_ps[:], zt, zt,
                            start=(c == 0 and t == 0),
                            stop=(c == NCHUNK - 1 and t == TC - 1),
                        )

                g_sb = gpool.tile([P, P], mybir.dt.float32, name=f"g_sb_{b}", tag="g")
                nc.vector.tensor_scalar_mul(g_sb[:], g_ps[:], INV_S)
                _shift_combine(nc, gpool, opool, psh, j64_sb, g_sb, out, b)

    nc.compile()
    return nc


def _build_nc_fp32():
    NCHUNK = 4
    TC = T // NCHUNK
    nc = bacc.Bacc("TRN2", target_bir_lowering=False, debug=False)

    xz = nc.dram_tensor(
        "xz", [BPC, NCHUNK, P, TC, D2], mybir.dt.float32, kind="ExternalInput"
    )
    j64 = nc.dram_tensor("j64", [P, D], mybir.dt.float32, kind="ExternalInput")
    out = nc.dram_tensor("out", [BPC, 2, D, D], mybir.dt.float32, kind="ExternalOutput")

    with tile.TileContext(nc) as tc:
        with (
            tc.tile_pool(name="consts", bufs=1) as consts,
            tc.tile_pool(name="zpool", bufs=3 * NCHUNK) as zpool,
            tc.tile_pool(name="gpool", bufs=2) as gpool,
            tc.tile_pool(name="opool", bufs=2) as opool,
            tc.tile_pool(name="psg", bufs=2, space="PSUM") as psg,
            tc.tile_pool(name="psh", bufs=2, space="PSUM") as psh,
        ):
            j64_sb = consts.tile([P, D], mybir.dt.float32)
            nc.scalar.dma_start(out=j64_sb[:], in_=j64[:])

            for b in range(BPC):
                zc = []
                for c in range(NCHUNK):
                    z = zpool.tile(
                        [P, TC, D2], mybir.dt.float32, name=f"z_{b}_{c}", tag="z"
                    )
                    nc.sync.dma_start(out=z[:], in_=xz[b, c])
                    zc.append(z)

                g_ps = psg.tile([P, P], mybir.dt.float32)
                for c in range(NCHUNK):
                    for t in range(TC):
                        zt = zc[c][:, t, :]
                        nc.tensor.matmul(
                            g_ps[:], zt, zt,
                            start=(c == 0 and t == 0),
                            stop=(c == NCHUNK - 1 and t == TC - 1),
                        )

                g_sb = gpool.tile([P, P], mybir.dt.float32, name=f"g_sb_{b}", tag="g")
                nc.vector.tensor_scalar_mul(g_sb[:], g_ps[:], INV_S)
                _shift_combine(nc, gpool, opool, psh, j64_sb, g_sb, out, b)

    nc.compile()
    return nc


def _build_nc_f32r_pair():
    NCHUNK = 4
    TC = T // NCHUNK
    nc = bacc.Bacc("TRN2", target_bir_lowering=False, debug=False)

    NP = BPC // 2
    xz = nc.dram_tensor(
        "xz", [NP, NCHUNK, P, TC, 2, D2], mybir.dt.float32r, kind="ExternalInput"
    )
    j64 = nc.dram_tensor("j64", [P, D], mybir.dt.float32, kind="ExternalInput")
    out = nc.dram_tensor("out", [BPC, 2, D, D], mybir.dt.float32, kind="ExternalOutput")

    with tile.TileContext(nc) as tc:
        with (
            tc.tile_pool(name="consts", bufs=1) as consts,
            tc.tile_pool(name="zpool", bufs=2 * NCHUNK) as zpool,
            tc.tile_pool(name="gpool", bufs=2) as gpool,
            tc.tile_pool(name="opool", bufs=2) as opool,
            tc.tile_pool(name="psg", bufs=4, space="PSUM") as psg,
            tc.tile_pool(name="psh", bufs=2, space="PSUM") as psh,
        ):
            j64_sb = consts.tile([P, D], mybir.dt.float32)
            nc.scalar.dma_start(out=j64_sb[:], in_=j64[:])

            for pair in range(NP):
                zc = []
                for c in range(NCHUNK):
                    z = zpool.tile(
                        [P, TC, 2, D2], mybir.dt.float32r,
                        name=f"z_{pair}_{c}", tag="z",
                    )
                    nc.sync.dma_start(out=z[:], in_=xz[pair, c])
                    zc.append(z)

                g_ps = [
                    psg.tile([P, 2 * P], mybir.dt.float32, name=f"g_ps_{pair}_{j}", tag="g")
                    for j in range(2)
                ]
                for c in range(NCHUNK):
                    for t in range(TC):
                        pair_rhs = zc[c][:, t, :, :]
                        first = c == 0 and t == 0
                        last = c == NCHUNK - 1 and t == TC - 1
                        for j in range(2):
                            nc.tensor.matmul(
                                g_ps[j][:],
                                zc[c][:, t, j, :],
                                pair_rhs,
                                start=first,
                                stop=last,
                            )

                for j in range(2):
                    b = 2 * pair + j
                    g_sb = gpool.tile([P, P], mybir.dt.float32, name=f"g_sb_{b}", tag="g")
                    nc.vector.tensor_scalar_mul(
                        g_sb[:], g_ps[j][:, j * P : (j + 1) * P], INV_S
                    )
                    _shift_combine(nc, gpool, opool, psh, j64_sb, g_sb, out, b)

    nc.compile()
    return nc


def _j64_host():
    j = np.zeros((P, D), np.float32)
    j[D + np.arange(D), np.arange(D)] = 1.0
    return j


def _chunk_view(a, nchunk):
    """[B, S, ...tail] -> [N_CORES, BPC, nchunk, P, T//nchunk, ...tail]."""
    tail = a.shape[2:]
    return a.reshape(N_CORES, BPC, nchunk, P, T // nchunk, *tail)


def _prep_fp16hl(xz):
    zh = xz.astype(np.float16)
    zl = ((xz - zh.astype(np.float32)) * LSCALE).astype(np.float16)
    zs = np.stack([zh, zl], axis=2)          # [B, S, 2, D2]
    return np.ascontiguousarray(_chunk_view(zs, 4))


def _prep_fp16(xz):
    return np.ascontiguousarray(_chunk_view(xz.astype(np.float16), 2))


def _prep_fp32(xz):
    return np.ascontiguousarray(_chunk_view(xz, 4))


def _prep_f32r_pair(xz):
    a = xz.reshape(N_CORES, BPC // 2, 2, 4, P, T // 4, D2)
    return np.ascontiguousarray(a.transpose(0, 1, 3, 4, 5, 2, 6))


_VARIANTS = {
    "fp16hl": (_build_nc_fp16hl, _prep_fp16hl, True),
    "fp16": (_build_nc_fp16, _prep_fp16, False),
    "fp32": (_build_nc_fp32, _prep_fp32, False),
    "f32r_pair": (_build_nc_f32r_pair, _prep_f32r_pair, False),
}


def kernel(input_real, input_imag):
    global LAST_RESULTS
    xr = np.asarray(input_real, dtype=np.float32)
    xi = np.asarray(input_imag, dtype=np.float32)
    assert xr.shape == (B, S, D) and xi.shape == (B, S, D)

    xz = np.concatenate([xr, xi], axis=2)  # [B, S, 2D]

    build, prep, needs_id = _VARIANTS[VARIANT]
    key = ("nc", VARIANT)
    if key not in _NC_CACHE:
        _NC_CACHE[key] = build()
    nc = _NC_CACHE[key]

    xz_cores = prep(xz)
    j64 = _j64_host()
    in_maps = []
    for c in range(N_CORES):
        m = {"xz": xz_cores[c], "j64": j64}
        if needs_id:
            m["id128"] = np.eye(P, dtype=np.float32)
        in_maps.append(m)
    tmpdir = os.environ.get("BASS_TMPDIR") or None
    res = run_bass_kernel_spmd(
        nc, in_maps, core_ids=list(range(N_CORES)), tmpdir=tmpdir
    )
    LAST_RESULTS = res

    outs = np.stack([res.results[c]["out"] for c in range(N_CORES)])  # [8,BPC,2,D,D]
    out = outs.reshape(B, 2, D, D)
    return np.ascontiguousarray(out[:, 0]), np.ascontiguousarray(out[:, 1])
